# revision 5
# baseline (speedup 1.0000x reference)
"""NonLocalBlock2D forward on 8 Trainium2 NeuronCores.

Math (per batch b of 4):
  xu = ImageUnshuffle(x[b])                      # [496, 4096]
  u = theta_w @ xu + theta_b                     # [32, 4096]  (theta over all positions i)
  v = phi_w   @ xu + phi_b                       # [32, 4096]
  g = g_w     @ xu + g_b                         # [32, 4096]
  S[i, j] = u[:, i] . v[:, j]
  f = softmax(S, axis=i)  (per-column stats)
  y[i, :] = sum_j f[i, j] * g[:, j]
  z = W_w @ y^T + W_b                            # [496, 4096]
  out[b] = ImageShuffle(z)

Sharding: 8 cores = (batch, j-half). Each core computes T = S^T[j, i] for its
2048 j columns and ALL 4096 i, exps once per element (ScalarE with accum_out
giving the softmax denominator Z_j for free), scales g^T by 1/Z_j, and
accumulates its partial y^T = sum_{j in half} (g_j/Z_j) E[j, :]. The final
W-projection is linear in y, so each core produces a partial z (with half the
bias) and the host sums the two per-batch partials during the gather.

Positions are permuted per-core so that the core's j-half occupies local
positions [0, 2048); the host un-permutes output columns.

All matmuls run as float32r (fp32 rounded to 11 mantissa bits, full PE rate).
"""

import sys

sys.path.insert(0, "/opt/trn_rl_repo")

import numpy as np
from contextlib import ExitStack

import concourse.bass as bass
import concourse.tile as tile
from concourse import bacc, mybir
from concourse.bass_utils import run_bass_kernel_spmd
from concourse.masks import make_identity

F32 = mybir.dt.float32
F32R = mybir.dt.float32r
EXP = mybir.ActivationFunctionType.Exp

N_BATCH = 4
C_IN = 31
SCALE = 4
H = W = 256
HS = WS = 64          # unshuffled spatial
NP = HS * WS          # 4096 positions
CIN = C_IN * SCALE * SCALE   # 496
CAUG = 512            # padded input channels (496 + ones row + zeros)
IC = 32               # inter channels
JH = NP // 2          # 2048 j positions per core
NJC = JH // 128       # 16 j-chunks of 128
N_SHARDS = 4          # shards of 4 j-chunks (pipelining granularity)
JPS = NJC // N_SHARDS  # 4 chunks per shard
NI512 = NP // 512     # 8 i-chunks of 512
OC = 4                # output-channel chunks of 124 (496 = 4*124)
OCW = 124


def round_f32r(x: np.ndarray) -> np.ndarray:
    """Round fp32 to the FP32R grid (11 mantissa bits, round half up)."""
    u = np.ascontiguousarray(x, dtype=np.float32).view(np.uint32).astype(np.uint64)
    q = ((u + (1 << 11)) >> 12 << 12) & 0xFFFFFFFF
    return q.astype(np.uint32).view(np.float32).reshape(x.shape)


def build_nc():
    nc = bacc.Bacc(None)

    # Per-core inputs (f32r pre-rounded by host).
    xu_d = nc.dram_tensor("xu", [4, 128, NP], F32R, kind="ExternalInput")
    wcat_d = nc.dram_tensor("wcat", [4, 128, 96], F32R, kind="ExternalInput")
    wproj_d = nc.dram_tensor("wproj", [32, CIN], F32R, kind="ExternalInput")
    wb_d = nc.dram_tensor("wb", [OC, OCW, 1], F32, kind="ExternalInput")
    # Output: partial z, tiled [o-chunk*8 + i-chunk, 124, 512].
    z_d = nc.dram_tensor("z", [OC * NI512, OCW, 512], F32, kind="ExternalOutput")

    with tile.TileContext(nc) as tc, ExitStack() as ctx:
        big = ctx.enter_context(tc.tile_pool(name="big", bufs=8))
        sb = ctx.enter_context(tc.tile_pool(name="sb", bufs=1))
        small = ctx.enter_context(tc.tile_pool(name="small", bufs=8))
        zpool = ctx.enter_context(tc.tile_pool(name="zpool", bufs=3))
        pmm = ctx.enter_context(tc.tile_pool(name="pmm", bufs=2, space="PSUM"))
        pyy = ctx.enter_context(tc.tile_pool(name="pyy", bufs=2, space="PSUM"))

        # ---- static tiles ----
        wcat_t = sb.tile([128, 4 * 96], F32R)     # k-chunk k at cols [96k, 96k+96)
        wproj_t = sb.tile([32, CIN], F32R)
        wb_t = sb.tile([OCW, OC], F32)            # W_b/2 chunk o at col o
        ident = sb.tile([32, 32], F32)
        u_t = sb.tile([32, NP], F32R)             # theta activations, all i
        v_t = sb.tile([32, JH], F32R)             # phi activations, local j
        g_t = sb.tile([32, JH], F32)              # g activations, local j
        gT_t = sb.tile([128, NJC * 32], F32)      # g^T, chunk jc at cols [32jc, 32jc+32)
        yaug_t = sb.tile([32, NP], F32R)          # y^T accumulator

        nc.sync.dma_start(out=wproj_t, in_=wproj_d[:, :])
        for k in range(4):
            nc.sync.dma_start(out=wcat_t[:, 96 * k:96 * (k + 1)], in_=wcat_d[k])
        for o in range(OC):
            nc.sync.dma_start(out=wb_t[:, o:o + 1], in_=wb_d[o])
        make_identity(nc, ident)

        # ---- xu load: 4 k-chunk tiles from the "big" pool, 2 DMAs each ----
        xu_tiles = []
        for k in range(4):
            t = big.tile([128, NP], F32R, tag="big")
            nc.sync.dma_start(out=t[:, 0:2048], in_=xu_d[k, :, 0:2048])
            nc.sync.dma_start(out=t[:, 2048:4096], in_=xu_d[k, :, 2048:4096])
            xu_tiles.append(t)

        # ---- conv: uvg[c, i] for local positions ----
        # i-chunks 0..3 (local j-half): full [theta|phi|g] (M=96)
        # i-chunks 4..7: theta only (M=32)
        for i in range(NI512):
            m = 96 if i < 4 else 32
            pt = pmm.tile([m, 512], F32, tag="mm")
            for k in range(4):
                nc.tensor.matmul(
                    pt,
                    wcat_t[:, 96 * k:96 * k + m],
                    xu_tiles[k][:, 512 * i:512 * (i + 1)],
                    start=(k == 0),
                    stop=(k == 3),
                )
            sl = slice(512 * i, 512 * (i + 1))
            nc.vector.tensor_copy(u_t[:, sl], pt[0:32, :])
            if i < 4:
                nc.vector.tensor_copy(v_t[:, sl], pt[32:64, :])
                nc.vector.tensor_copy(g_t[:, sl], pt[64:96, :])

        # ---- attention, software-pipelined by shard ----
        E_tiles = [None] * NJC     # exp(T) per j-chunk, [128 j, 4096 i]
        gsc_tiles = [None] * NJC   # (g^T / Z) per j-chunk, [128 j, 32 c]

        def phase1_chunk(jc):
            """T = v_chunk^T @ u -> exp -> Z -> gsc for j-chunk jc."""
            # transpose g chunk: [32, 128] -> [128, 32]
            ptr = pmm.tile([128, 32], F32, tag="mm")
            nc.tensor.transpose(ptr, g_t[:, 128 * jc:128 * (jc + 1)], ident)
            gsl = slice(32 * jc, 32 * (jc + 1))
            nc.vector.tensor_copy(gT_t[:, gsl], ptr)

            E = big.tile([128, NP], F32R, tag="big")
            zparts = []
            for off, ln in ((0, 1536), (1536, 1536), (3072, 1024)):
                pt = pmm.tile([128, ln], F32, tag="mm")
                for ii in range(ln // 512):
                    nc.tensor.matmul(
                        pt[:, 512 * ii:512 * (ii + 1)],
                        v_t[:, 128 * jc:128 * (jc + 1)],
                        u_t[:, off + 512 * ii:off + 512 * (ii + 1)],
                        start=True,
                        stop=True,
                    )
                zp = small.tile([128, 1], F32, tag="zp")
                nc.scalar.activation(E[:, off:off + ln], pt, EXP, accum_out=zp)
                zparts.append(zp)
            zs = small.tile([128, 1], F32, tag="zs")
            nc.vector.tensor_add(zs, zparts[0], zparts[1])
            nc.vector.tensor_add(zs, zs, zparts[2])
            rz = small.tile([128, 1], F32, tag="rz")
            nc.vector.reciprocal(rz, zs)
            gsc = small.tile([128, 32], F32R, tag="gsc")
            nc.vector.tensor_scalar_mul(gsc, gT_t[:, gsl], rz)
            E_tiles[jc] = E
            gsc_tiles[jc] = gsc

        def phase2_sess(s, sess):
            """y^T[:, sess] += sum over shard s's 4 j-chunks."""
            py = pyy.tile([32, 512], F32, tag="y")
            isl = slice(512 * sess, 512 * (sess + 1))
            for q in range(JPS):
                jc = JPS * s + q
                nc.tensor.matmul(
                    py,
                    gsc_tiles[jc],
                    E_tiles[jc][:, isl],
                    start=(q == 0),
                    stop=(q == JPS - 1),
                )
            if s == 0:
                nc.vector.tensor_copy(yaug_t[0:32, isl], py)
            else:
                nc.vector.tensor_add(yaug_t[0:32, isl], yaug_t[0:32, isl], py)
            if s == N_SHARDS - 1:
                wproj_chunk(sess)

        def wproj_chunk(i):
            """z[:, i-chunk] = W_aug @ y_aug[:, i-chunk]."""
            for o in range(OC):
                pz = pmm.tile([OCW, 512], F32, tag="mm")
                nc.tensor.matmul(
                    pz,
                    wproj_t[:, OCW * o:OCW * (o + 1)],
                    yaug_t[:, 512 * i:512 * (i + 1)],
                    start=True,
                    stop=True,
                )
                zt = zpool.tile([OCW, 512], F32, tag="z")
                nc.vector.tensor_scalar_add(zt, pz, wb_t[:, o:o + 1])
                nc.sync.dma_start(out=z_d[o * NI512 + i], in_=zt)

        # pipeline: shard s phase1 overlaps shard s-1 phase2 (on ScalarE vs PE)
        for s in range(N_SHARDS):
            for q in range(JPS):
                phase1_chunk(JPS * s + q)
                if s > 0:
                    phase2_sess(s - 1, 2 * q)
                    phase2_sess(s - 1, 2 * q + 1)
        for sess in range(NI512):
            phase2_sess(N_SHARDS - 1, sess)

    nc.finalize()
    return nc


_NC_CACHE = None


def _get_nc():
    global _NC_CACHE
    if _NC_CACHE is None:
        _NC_CACHE = build_nc()
    return _NC_CACHE


def _prep_inputs(x, g_w, g_b, theta_w, theta_b, phi_w, phi_b, W_w, W_b):
    # ImageUnshuffle: [4, 31, 256, 256] -> [4, 496, 4096]
    xu = (
        x.reshape(N_BATCH, C_IN, HS, SCALE, WS, SCALE)
        .transpose(0, 3, 5, 1, 2, 4)
        .reshape(N_BATCH, CIN, NP)
    )

    wcat = np.zeros((CAUG, 96), np.float32)
    wcat[:CIN, 0:32] = theta_w.T
    wcat[:CIN, 32:64] = phi_w.T
    wcat[:CIN, 64:96] = g_w.T
    wcat[CIN, 0:32] = theta_b
    wcat[CIN, 32:64] = phi_b
    wcat[CIN, 64:96] = g_b
    wcat = round_f32r(wcat).reshape(4, 128, 96)

    wproj = round_f32r(W_w.T.astype(np.float32))
    wb = (W_b * 0.5).astype(np.float32).reshape(OC, OCW, 1)

    in_maps = []
    for c in range(8):
        b, jh = divmod(c, 2)
        xc = np.empty((CAUG, NP), np.float32)
        if jh == 0:
            xc[:CIN] = xu[b]
        else:
            xc[:CIN, 0:JH] = xu[b][:, JH:]
            xc[:CIN, JH:] = xu[b][:, 0:JH]
        xc[CIN] = 1.0
        xc[CIN + 1:] = 0.0
        in_maps.append(
            {
                "xu": round_f32r(xc).reshape(4, 128, NP),
                "wcat": wcat,
                "wproj": wproj,
                "wb": wb,
            }
        )
    return in_maps


def _assemble(results):
    out = np.empty((N_BATCH, C_IN, H, W), np.float32)
    for b in range(N_BATCH):
        zsum = None
        for jh in (0, 1):
            zt = results[2 * b + jh]["z"]  # [32, 124, 512]
            zc = np.empty((CIN, NP), np.float32)
            for o in range(OC):
                for i in range(NI512):
                    zc[o * OCW:(o + 1) * OCW, 512 * i:512 * (i + 1)] = zt[o * NI512 + i]
            if jh == 1:  # un-permute columns (halves were swapped)
                zc = np.concatenate([zc[:, JH:], zc[:, 0:JH]], axis=1)
            zsum = zc if zsum is None else zsum + zc
        # ImageShuffle: [496, 4096] -> [31, 256, 256]
        out[b] = (
            zsum.reshape(SCALE, SCALE, C_IN, HS, WS)
            .transpose(2, 3, 0, 4, 1)
            .reshape(C_IN, H, W)
        )
    return out


def kernel(**inputs) -> np.ndarray:
    nc = _get_nc()
    in_maps = _prep_inputs(**{k: np.asarray(v) for k, v in inputs.items()})
    res = run_bass_kernel_spmd(nc, in_maps, core_ids=list(range(8)))
    return _assemble(res.results)


def _install_trace_hooks():
    """Register the antenv.axon_hooks NTFF hook (missing on this image) and
    stub out the artifact upload. Test-harness only."""
    import types, ctypes, contextlib

    if "antenv.axon_hooks" not in sys.modules:
        so_path = "/opt/axon/libaxon_pjrt.so"
        lib = ctypes.CDLL(so_path)
        lib.axon_start_nrt_profile.argtypes = [
            ctypes.POINTER(ctypes.c_int64),
            ctypes.c_size_t,
        ]
        lib.axon_start_nrt_profile.restype = ctypes.c_int64
        lib.axon_stop_nrt_profile.argtypes = [ctypes.c_char_p]
        lib.axon_stop_nrt_profile.restype = ctypes.c_int64

        @contextlib.contextmanager
        def _hook(output_dir, device_ids):
            import jax

            jax.devices()
            if device_ids:
                ids = (ctypes.c_int64 * len(device_ids))(*device_ids)
                rc = lib.axon_start_nrt_profile(ids, len(device_ids))
            else:
                rc = lib.axon_start_nrt_profile(None, 0)
            if rc != 0:
                raise RuntimeError(f"axon_start_nrt_profile rc={rc}")
            try:
                yield
            finally:
                n = lib.axon_stop_nrt_profile(str(output_dir).encode())
                print(f"profile: {n} ntff file(s) written to {output_dir}")

        mod = types.ModuleType("antenv.axon_hooks")
        mod.get_axon_ntff_profile_hook = lambda: _hook
        mod.set_axon_ntff_profile_hook = lambda h: None
        sys.modules["antenv.axon_hooks"] = mod

    import concourse.bass_utils as bu

    bu.upload_artifacts = lambda tmpdir: f"local://{tmpdir}"


def run_traced(**inputs):
    """Like kernel() but with NTFF tracing; returns (output, BassKernelResults)."""
    _install_trace_hooks()
    nc = _get_nc()
    in_maps = _prep_inputs(**{k: np.asarray(v) for k, v in inputs.items()})
    res = run_bass_kernel_spmd(
        nc, in_maps, core_ids=list(range(8)), trace=True, tmpdir="/tmp/ntff_trace"
    )
    return _assemble(res.results), res


# revision 7
# speedup vs baseline: 1.0198x; 1.0198x over previous
"""NonLocalBlock2D forward on 8 Trainium2 NeuronCores.

Math (per batch b of 4):
  xu = ImageUnshuffle(x[b])                      # [496, 4096]
  u = theta_w @ xu + theta_b                     # [32, 4096]  (theta over all positions i)
  v = phi_w   @ xu + phi_b                       # [32, 4096]
  g = g_w     @ xu + g_b                         # [32, 4096]
  S[i, j] = u[:, i] . v[:, j]
  f = softmax(S, axis=i)  (per-column stats)
  y[i, :] = sum_j f[i, j] * g[:, j]
  z = W_w @ y^T + W_b                            # [496, 4096]
  out[b] = ImageShuffle(z)

Sharding: 8 cores = (batch, j-half). Each core computes T = S^T[j, i] for its
2048 j columns and ALL 4096 i, exps once per element (ScalarE with accum_out
giving the softmax denominator Z_j for free), scales g^T by 1/Z_j, and
accumulates its partial y^T = sum_{j in half} (g_j/Z_j) E[j, :]. The final
W-projection is linear in y, so each core produces a partial z (with half the
bias) and the host sums the two per-batch partials during the gather.

Positions are permuted per-core so that the core's j-half occupies local
positions [0, 2048); the host un-permutes output columns.

All matmuls run as float32r (fp32 rounded to 11 mantissa bits, full PE rate).
"""

import sys

sys.path.insert(0, "/opt/trn_rl_repo")

import numpy as np
from contextlib import ExitStack

import concourse.bass as bass
import concourse.tile as tile
from concourse import bacc, mybir
from concourse.bass_utils import run_bass_kernel_spmd
from concourse.masks import make_identity

F32 = mybir.dt.float32
F32R = mybir.dt.float32r
BF16 = mybir.dt.bfloat16
EXP = mybir.ActivationFunctionType.Exp

# Walrus ships with --enable-ldw-opt=false; our matmuls reuse the same
# stationary operand across long runs, and the dedup is a large win.
import concourse.bass_utils as _bu

if not getattr(_bu, "_ldw_opt_patched", False):
    _orig_run_command = _bu.run_command

    def _run_command_ldwopt(argv, **kwargs):
        if isinstance(argv, list):
            argv = [
                a
                for a in argv
            ]
        return _orig_run_command(argv, **kwargs)

    _bu.run_command = _run_command_ldwopt
    _bu._ldw_opt_patched = True

N_BATCH = 4
C_IN = 31
SCALE = 4
H = W = 256
HS = WS = 64          # unshuffled spatial
NP = HS * WS          # 4096 positions
CIN = C_IN * SCALE * SCALE   # 496
CAUG = 512            # padded input channels (496 + ones row + zeros)
IC = 32               # inter channels
JH = NP // 2          # 2048 j positions per core
NJC = JH // 128       # 16 j-chunks of 128
N_SHARDS = 4          # shards of 4 j-chunks (pipelining granularity)
JPS = NJC // N_SHARDS  # 4 chunks per shard
NI512 = NP // 512     # 8 i-chunks of 512
OC = 4                # output-channel chunks of 124 (496 = 4*124)
OCW = 124


def round_f32r(x: np.ndarray) -> np.ndarray:
    """Round fp32 to the FP32R grid (11 mantissa bits, round half up)."""
    u = np.ascontiguousarray(x, dtype=np.float32).view(np.uint32).astype(np.uint64)
    q = ((u + (1 << 11)) >> 12 << 12) & 0xFFFFFFFF
    return q.astype(np.uint32).view(np.float32).reshape(x.shape)


def build_nc():
    nc = bacc.Bacc(None)

    # Per-core inputs (f32r pre-rounded by host).
    xu_d = nc.dram_tensor("xu", [4, 128, NP], F32R, kind="ExternalInput")
    wcat_d = nc.dram_tensor("wcat", [4, 128, 96], F32R, kind="ExternalInput")
    wproj_d = nc.dram_tensor("wproj", [32, CIN], F32R, kind="ExternalInput")
    wb_d = nc.dram_tensor("wb", [OC, OCW, 1], F32, kind="ExternalInput")
    # Output: partial z, tiled [o-chunk*8 + i-chunk, 124, 512].
    z_d = nc.dram_tensor("z", [OC * NI512, OCW, 512], F32, kind="ExternalOutput")

    with tile.TileContext(nc) as tc, ExitStack() as ctx:
        big = ctx.enter_context(tc.tile_pool(name="big", bufs=12))
        sb = ctx.enter_context(tc.tile_pool(name="sb", bufs=1))
        small = ctx.enter_context(tc.tile_pool(name="small", bufs=8))
        zpool = ctx.enter_context(tc.tile_pool(name="zpool", bufs=3))
        pmm = ctx.enter_context(tc.tile_pool(name="pmm", bufs=2, space="PSUM"))
        pyy = ctx.enter_context(tc.tile_pool(name="pyy", bufs=2, space="PSUM"))

        # ---- static tiles ----
        wcat_t = sb.tile([128, 4 * 96], F32R)     # k-chunk k at cols [96k, 96k+96)
        wproj_t = sb.tile([32, CIN], F32R)
        wb_t = sb.tile([OCW, OC], F32)            # W_b/2 chunk o at col o
        ident = sb.tile([32, 32], F32)
        u_t = sb.tile([32, NP], F32R)             # theta activations, all i
        v_t = sb.tile([32, JH], F32R)             # phi activations, local j
        g_t = sb.tile([32, JH], F32)              # g activations, local j
        gT_t = sb.tile([128, NJC * 32], F32)      # g^T, chunk jc at cols [32jc, 32jc+32)
        yaug_t = sb.tile([32, NP], F32R)          # y^T accumulator

        nc.sync.dma_start(out=wproj_t, in_=wproj_d[:, :])
        for k in range(4):
            nc.sync.dma_start(out=wcat_t[:, 96 * k:96 * (k + 1)], in_=wcat_d[k])
        for o in range(OC):
            nc.sync.dma_start(out=wb_t[:, o:o + 1], in_=wb_d[o])
        make_identity(nc, ident)

        # ---- xu load: 8 half-tiles [128, 2048] from the "big" pool ----
        xu_tiles = []
        for k in range(4):
            for h in range(2):
                t = big.tile([128, 2048], F32R, tag="big")
                nc.sync.dma_start(out=t, in_=xu_d[k, :, 2048 * h:2048 * (h + 1)])
                xu_tiles.append(t)

        # ---- conv: uvg[c, i] for local positions ----
        # i-chunks 0..3 (local j-half): full [theta|phi|g] (M=96)
        # i-chunks 4..7: theta only (M=32)
        for i in range(NI512):
            m = 96 if i < 4 else 32
            pt = pmm.tile([m, 512], F32, tag="mm")
            for k in range(4):
                nc.tensor.matmul(
                    pt,
                    wcat_t[:, 96 * k:96 * k + m],
                    xu_tiles[2 * k + (i >= 4)][:, 512 * (i % 4):512 * (i % 4 + 1)],
                    start=(k == 0),
                    stop=(k == 3),
                )
            sl = slice(512 * i, 512 * (i + 1))
            nc.vector.tensor_copy(u_t[:, sl], pt[0:32, :])
            if i < 4:
                nc.vector.tensor_copy(v_t[:, sl], pt[32:64, :])
                nc.vector.tensor_copy(g_t[:, sl], pt[64:96, :])

        # ---- attention, software-pipelined by shard ----
        E_tiles = [None] * NJC     # exp(T) per j-chunk, [128 j, 4096 i]
        gsc_tiles = [None] * NJC   # (g^T / Z) per j-chunk, [128 j, 32 c]

        def phase1_chunk(jc):
            """T = v_chunk^T @ u -> exp -> Z -> gsc for j-chunk jc."""
            # transpose g chunk: [32, 128] -> [128, 32]
            ptr = pmm.tile([128, 32], F32, tag="mm")
            nc.tensor.transpose(ptr, g_t[:, 128 * jc:128 * (jc + 1)], ident)
            gsl = slice(32 * jc, 32 * (jc + 1))
            nc.vector.tensor_copy(gT_t[:, gsl], ptr)

            E = big.tile([128, NP], BF16, tag="big")
            zparts = []
            for off, ln in ((0, 1536), (1536, 1536), (3072, 1024)):
                pt = pmm.tile([128, ln], F32, tag="mm")
                for ii in range(ln // 512):
                    nc.tensor.matmul(
                        pt[:, 512 * ii:512 * (ii + 1)],
                        v_t[:, 128 * jc:128 * (jc + 1)],
                        u_t[:, off + 512 * ii:off + 512 * (ii + 1)],
                        start=True,
                        stop=True,
                    )
                zp = small.tile([128, 1], F32, tag="zp")
                nc.scalar.activation(E[:, off:off + ln], pt, EXP, accum_out=zp)
                zparts.append(zp)
            zs = small.tile([128, 1], F32, tag="zs")
            nc.vector.tensor_add(zs, zparts[0], zparts[1])
            nc.vector.tensor_add(zs, zs, zparts[2])
            rz = small.tile([128, 1], F32, tag="rz")
            nc.vector.reciprocal(rz, zs)
            gsc = small.tile([128, 32], BF16, tag="gsc")
            nc.vector.tensor_scalar_mul(gsc, gT_t[:, gsl], rz)
            E_tiles[jc] = E
            gsc_tiles[jc] = gsc

        def phase2_sess(s, sess):
            """y^T[:, sess] += sum over shard s's 4 j-chunks."""
            py = pyy.tile([32, 512], F32, tag="y")
            isl = slice(512 * sess, 512 * (sess + 1))
            for q in range(JPS):
                jc = JPS * s + q
                nc.tensor.matmul(
                    py,
                    gsc_tiles[jc],
                    E_tiles[jc][:, isl],
                    start=(q == 0),
                    stop=(q == JPS - 1),
                )
            if s == 0:
                nc.vector.tensor_copy(yaug_t[0:32, isl], py)
            else:
                nc.vector.tensor_add(yaug_t[0:32, isl], yaug_t[0:32, isl], py)
            if s == N_SHARDS - 1:
                wproj_chunk(sess)

        def wproj_chunk(i):
            """z[:, i-chunk] = W_aug @ y_aug[:, i-chunk]."""
            for o in range(OC):
                pz = pmm.tile([OCW, 512], F32, tag="mm")
                nc.tensor.matmul(
                    pz,
                    wproj_t[:, OCW * o:OCW * (o + 1)],
                    yaug_t[:, 512 * i:512 * (i + 1)],
                    start=True,
                    stop=True,
                )
                zt = zpool.tile([OCW, 512], F32, tag="z")
                nc.vector.tensor_scalar_add(zt, pz, wb_t[:, o:o + 1])
                nc.sync.dma_start(out=z_d[o * NI512 + i], in_=zt)

        # pipeline: shard s phase1 overlaps shard s-1 phase2 (on ScalarE vs PE)
        for s in range(N_SHARDS):
            for q in range(JPS):
                phase1_chunk(JPS * s + q)
                if s > 0:
                    phase2_sess(s - 1, 2 * q)
                    phase2_sess(s - 1, 2 * q + 1)
        for sess in range(NI512):
            phase2_sess(N_SHARDS - 1, sess)

    nc.finalize()
    return nc


_NC_CACHE = None


def _get_nc():
    global _NC_CACHE
    if _NC_CACHE is None:
        _NC_CACHE = build_nc()
    return _NC_CACHE


def _prep_inputs(x, g_w, g_b, theta_w, theta_b, phi_w, phi_b, W_w, W_b):
    # ImageUnshuffle: [4, 31, 256, 256] -> [4, 496, 4096]
    xu = (
        x.reshape(N_BATCH, C_IN, HS, SCALE, WS, SCALE)
        .transpose(0, 3, 5, 1, 2, 4)
        .reshape(N_BATCH, CIN, NP)
    )

    wcat = np.zeros((CAUG, 96), np.float32)
    wcat[:CIN, 0:32] = theta_w.T
    wcat[:CIN, 32:64] = phi_w.T
    wcat[:CIN, 64:96] = g_w.T
    wcat[CIN, 0:32] = theta_b
    wcat[CIN, 32:64] = phi_b
    wcat[CIN, 64:96] = g_b
    wcat = round_f32r(wcat).reshape(4, 128, 96)

    wproj = round_f32r(W_w.T.astype(np.float32))
    wb = (W_b * 0.5).astype(np.float32).reshape(OC, OCW, 1)

    in_maps = []
    for c in range(8):
        b, jh = divmod(c, 2)
        xc = np.empty((CAUG, NP), np.float32)
        if jh == 0:
            xc[:CIN] = xu[b]
        else:
            xc[:CIN, 0:JH] = xu[b][:, JH:]
            xc[:CIN, JH:] = xu[b][:, 0:JH]
        xc[CIN] = 1.0
        xc[CIN + 1:] = 0.0
        in_maps.append(
            {
                "xu": round_f32r(xc).reshape(4, 128, NP),
                "wcat": wcat,
                "wproj": wproj,
                "wb": wb,
            }
        )
    return in_maps


def _assemble(results):
    out = np.empty((N_BATCH, C_IN, H, W), np.float32)
    for b in range(N_BATCH):
        zsum = None
        for jh in (0, 1):
            zt = results[2 * b + jh]["z"]  # [32, 124, 512]
            zc = np.empty((CIN, NP), np.float32)
            for o in range(OC):
                for i in range(NI512):
                    zc[o * OCW:(o + 1) * OCW, 512 * i:512 * (i + 1)] = zt[o * NI512 + i]
            if jh == 1:  # un-permute columns (halves were swapped)
                zc = np.concatenate([zc[:, JH:], zc[:, 0:JH]], axis=1)
            zsum = zc if zsum is None else zsum + zc
        # ImageShuffle: [496, 4096] -> [31, 256, 256]
        out[b] = (
            zsum.reshape(SCALE, SCALE, C_IN, HS, WS)
            .transpose(2, 3, 0, 4, 1)
            .reshape(C_IN, H, W)
        )
    return out


def kernel(**inputs) -> np.ndarray:
    nc = _get_nc()
    in_maps = _prep_inputs(**{k: np.asarray(v) for k, v in inputs.items()})
    res = run_bass_kernel_spmd(nc, in_maps, core_ids=list(range(8)))
    return _assemble(res.results)


def _install_trace_hooks():
    """Register the antenv.axon_hooks NTFF hook (missing on this image) and
    stub out the artifact upload. Test-harness only."""
    import types, ctypes, contextlib

    if "antenv.axon_hooks" not in sys.modules:
        so_path = "/opt/axon/libaxon_pjrt.so"
        lib = ctypes.CDLL(so_path)
        lib.axon_start_nrt_profile.argtypes = [
            ctypes.POINTER(ctypes.c_int64),
            ctypes.c_size_t,
        ]
        lib.axon_start_nrt_profile.restype = ctypes.c_int64
        lib.axon_stop_nrt_profile.argtypes = [ctypes.c_char_p]
        lib.axon_stop_nrt_profile.restype = ctypes.c_int64

        @contextlib.contextmanager
        def _hook(output_dir, device_ids):
            import jax

            jax.devices()
            if device_ids:
                ids = (ctypes.c_int64 * len(device_ids))(*device_ids)
                rc = lib.axon_start_nrt_profile(ids, len(device_ids))
            else:
                rc = lib.axon_start_nrt_profile(None, 0)
            if rc != 0:
                raise RuntimeError(f"axon_start_nrt_profile rc={rc}")
            try:
                yield
            finally:
                n = lib.axon_stop_nrt_profile(str(output_dir).encode())
                print(f"profile: {n} ntff file(s) written to {output_dir}")

        mod = types.ModuleType("antenv.axon_hooks")
        mod.get_axon_ntff_profile_hook = lambda: _hook
        mod.set_axon_ntff_profile_hook = lambda h: None
        sys.modules["antenv.axon_hooks"] = mod

    import concourse.bass_utils as bu

    bu.upload_artifacts = lambda tmpdir: f"local://{tmpdir}"


def run_traced(**inputs):
    """Like kernel() but with NTFF tracing; returns (output, BassKernelResults)."""
    _install_trace_hooks()
    nc = _get_nc()
    in_maps = _prep_inputs(**{k: np.asarray(v) for k, v in inputs.items()})
    res = run_bass_kernel_spmd(
        nc, in_maps, core_ids=list(range(8)), trace=True, tmpdir="/tmp/ntff_trace"
    )
    return _assemble(res.results), res
